# revision 12
# baseline (speedup 1.0000x reference)
"""Trainium2 Bass kernel for nn_GatedMLPConcat (MoE-routed gated MLP).

Math (reference):
  out_straight = relu(x @ W1s.T + b1s)                    # [N, 1024]
  out_gated    = relu(x @ W1g[gid].T + b1g[gid])          # [N, 512]  (only the selected expert matters)
  h  = relu(concat([out_straight, out_gated]) @ W2.T + b2)  # [N, 512]
  out = h @ W3.T                                           # [N, 1024]

Strategy:
  - Host-side MoE routing: group samples by gate_id into dense per-expert
    column blocks, avoiding the reference's 32x wasted expert FLOPs.
  - Load-balanced expert slots: experts sorted by sample count; rank 8j+i
    goes to core i, slot j, so all 8 cores share identical slot widths
    S[j] = max count in octile j (~[296,264,256,248] for seed 0, ~4% padding
    vs 16% for a uniform capacity). SPMD: one program, per-core weight data.
  - Everything on device is feature-major ([feature, sample] = matmul's
    [M partition, N free]); the host pre-transposes weights and activations.
  - bf16 matmul operands, f32 PSUM accumulation, f32 output.
  - DMAs in consumption order on the sync FIFO ring: x(kt0,slot0) first so
    the first matmul can issue ~1.5us after DMA start, then x/w1s per-kt
    interleaved, w1g per expert, w2, w3. Bias rides the scalar-engine ring
    (a tiny descriptor-bound transfer would stall the main ring).
  - L2/L3 interleaved per column slot; output DMA per quarter overlaps
    compute and keeps the tail short.
"""

import numpy as np

import concourse.bacc as bacc
import concourse.bass as bass
import concourse.mybir as mybir
from concourse import tile
from concourse.bass_utils import run_bass_kernel_spmd

# Problem constants (hardcoded per contract)
N = 8192
IN_DIM = 1024
D = 512
G = 32
OUT = 1024
SM, GM = 2, 1
SD = SM * D          # 1024 straight features
CD = (SM + GM) * D   # 1536 concat features

N_CORES = 8
EPC = G // N_CORES   # 4 expert slots per core

KT1 = IN_DIM // 128  # 8  k-tiles for layer 1
FT1 = SD // 128      # 8  straight feature tiles
FTG = D // 128       # 4  gated feature tiles (per expert)
KT2 = CD // 128      # 12 k-tiles for layer 2
FT2 = D // 128       # 4  layer-2 feature tiles
KT3 = D // 128       # 4  k-tiles for layer 3
FT3 = OUT // 128     # 8  layer-3 feature tiles

BF16 = mybir.dt.bfloat16
F32 = mybir.dt.float32
NP_BF16 = mybir.dt.np(BF16)


def _build_bass(slots):
    S = list(slots)            # per-slot column widths, identical on all cores
    OFF = [0]
    for s in S:
        OFF.append(OFF[-1] + s)
    cols = OFF[-1]
    smax = S[0]

    nc = bacc.Bacc()

    x_d = nc.dram_tensor("x", [IN_DIM, cols], BF16, kind="ExternalInput")
    w1s_d = nc.dram_tensor("w1s", [IN_DIM, SD], BF16, kind="ExternalInput")
    # expert-slot-major: [e][kt][128][D]
    w1g_d = nc.dram_tensor("w1g", [EPC * IN_DIM, D], BF16, kind="ExternalInput")
    w2_d = nc.dram_tensor("w2", [CD, D], BF16, kind="ExternalInput")
    w3_d = nc.dram_tensor("w3", [D, OUT], BF16, kind="ExternalInput")
    bias_d = nc.dram_tensor("bias", [128, FT1 + EPC * FTG + FT2], F32,
                            kind="ExternalInput")
    # [ct][ft][128][smax]; cols beyond S[ct] unwritten
    out_d = nc.dram_tensor("out", [EPC * OUT, smax], F32, kind="ExternalOutput")

    with tile.TileContext(nc) as tc:
        with (
            tc.tile_pool(name="acts", bufs=1) as acts,
            tc.tile_pool(name="weights", bufs=1) as weights,
            tc.tile_pool(name="outp", bufs=3) as outp,
            tc.tile_pool(name="psum", bufs=8, space="PSUM") as psum,
        ):
            x_sb = acts.tile([128, KT1 * cols], BF16)       # x[kt][:, col]
            h1_sb = acts.tile([128, KT2 * cols], BF16)      # concat acts
            h2_sb = acts.tile([128, KT3 * cols], BF16)
            w1s_sb = weights.tile([128, KT1 * SD], BF16)    # w1s[kt][:, feat]
            w1g_sb = weights.tile([128, EPC * KT1 * D], BF16)  # [e][kt][feat]
            w2_sb = weights.tile([128, KT2 * D], BF16)
            w3_sb = weights.tile([128, KT3 * OUT], BF16)
            bias_sb = weights.tile([128, FT1 + EPC * FTG + FT2], F32)

            # --- PE prewarm: junk matmuls with no DMA deps keep the PE HAM
            # activity window busy during the input lead-in, so real matmuls
            # start at 2.4GHz instead of the cold 1.2GHz gate
            warm_sb = weights.tile([128, 128], BF16)
            nc.gpsimd.memset(warm_sb[:], 0.0)
            for w in range(32):
                wp = psum.tile([128, 128], F32, tag="ps", name=f"warm{w}")
                nc.tensor.matmul(wp[:], warm_sb[:], warm_sb[:],
                                 start=True, stop=True)

            # --- DMAs in consumption order (FIFO ring on sync) ---
            nc.scalar.dma_start(bias_sb[:], bias_d[:])
            # first matmul needs only x(kt0, slot0) + w1s(kt0, ft0)
            nc.sync.dma_start(x_sb[:, 0:S[0]], x_d[0:128, 0:S[0]])
            nc.sync.dma_start(w1s_sb[:, 0:128], w1s_d[0:128, 0:128])
            nc.sync.dma_start(w1s_sb[:, 128:SD], w1s_d[0:128, 128:])
            nc.sync.dma_start(x_sb[:, S[0]:cols], x_d[0:128, S[0]:])
            for kt in range(1, KT1):
                nc.sync.dma_start(
                    x_sb[:, kt * cols:(kt + 1) * cols],
                    x_d[kt * 128:(kt + 1) * 128, :],
                )
                nc.sync.dma_start(
                    w1s_sb[:, kt * SD:(kt + 1) * SD],
                    w1s_d[kt * 128:(kt + 1) * 128, :],
                )
            for e in range(EPC):
                nc.sync.dma_start(
                    w1g_sb[:, e * KT1 * D:(e + 1) * KT1 * D].rearrange(
                        "p (k c) -> p k c", k=KT1),
                    w1g_d[e * IN_DIM:(e + 1) * IN_DIM, :].rearrange(
                        "(k p) c -> p k c", p=128),
                )
            nc.sync.dma_start(
                w2_sb[:].rearrange("p (k c) -> p k c", k=KT2),
                w2_d[:].rearrange("(k p) c -> p k c", p=128))
            nc.sync.dma_start(
                w3_sb[:].rearrange("p (k c) -> p k c", k=KT3),
                w3_d[:].rearrange("(k p) c -> p k c", p=128))

            def xs(kt, ct):
                return x_sb[:, kt * cols + OFF[ct]: kt * cols + OFF[ct + 1]]

            def h1s(ft, ct):
                return h1_sb[:, ft * cols + OFF[ct]: ft * cols + OFF[ct + 1]]

            def h2s(ft, ct):
                return h2_sb[:, ft * cols + OFF[ct]: ft * cols + OFF[ct + 1]]

            # ---- Layer 1, straight path: h1[0:8] = relu(W1s @ x + b1s)
            for ft in range(FT1):
                ps = [psum.tile([128, S[i]], F32, tag="ps", name=f"ps{ft}_{i}")
                      for i in range(EPC)]
                for kt in range(KT1):
                    w_ap = w1s_sb[:, kt * SD + ft * 128: kt * SD + (ft + 1) * 128]
                    for ct in range(EPC):
                        nc.tensor.matmul(
                            ps[ct][:], w_ap, xs(kt, ct),
                            start=(kt == 0), stop=(kt == KT1 - 1),
                        )
                for ct in range(EPC):
                    nc.scalar.activation(
                        h1s(ft, ct), ps[ct][:],
                        mybir.ActivationFunctionType.Relu,
                        bias=bias_sb[:, ft:ft + 1],
                    )

            # ---- Layer 1, gated path: slot e columns get expert e's features
            for e in range(EPC):
                for ft in range(FTG):
                    p = psum.tile([128, S[e]], F32, tag="ps", name=f"psg{e}_{ft}")
                    for kt in range(KT1):
                        col0 = e * KT1 * D + kt * D + ft * 128
                        w_ap = w1g_sb[:, col0: col0 + 128]
                        nc.tensor.matmul(
                            p[:], w_ap, xs(kt, e),
                            start=(kt == 0), stop=(kt == KT1 - 1),
                        )
                    nc.scalar.activation(
                        h1s(FT1 + ft, e), p[:],
                        mybir.ActivationFunctionType.Relu,
                        bias=bias_sb[:, FT1 + e * FTG + ft:
                                     FT1 + e * FTG + ft + 1],
                    )

            # ---- Layers 2+3 interleaved per column slot; out DMA per quarter
            for ct in range(EPC):
                sw = S[ct]
                ps2 = [psum.tile([128, sw], F32, tag="ps", name=f"ps2_{ct}_{i}")
                       for i in range(FT2)]
                for kt in range(KT2):
                    for ft in range(FT2):
                        w_ap = w2_sb[:, kt * D + ft * 128: kt * D + (ft + 1) * 128]
                        nc.tensor.matmul(
                            ps2[ft][:], w_ap, h1s(kt, ct),
                            start=(kt == 0), stop=(kt == KT2 - 1),
                        )
                for ft in range(FT2):
                    nc.scalar.activation(
                        h2s(ft, ct), ps2[ft][:],
                        mybir.ActivationFunctionType.Relu,
                        bias=bias_sb[:, FT1 + EPC * FTG + ft:
                                     FT1 + EPC * FTG + ft + 1],
                    )

                o_sb = outp.tile([128, FT3 * sw], F32, tag="o", name=f"o{ct}")
                for ft in range(FT3):
                    p3 = psum.tile([128, sw], F32, tag="ps", name=f"ps3_{ct}_{ft}")
                    for kt in range(KT3):
                        w_ap = w3_sb[:, kt * OUT + ft * 128: kt * OUT + (ft + 1) * 128]
                        nc.tensor.matmul(
                            p3[:], w_ap, h2s(kt, ct),
                            start=(kt == 0), stop=(kt == KT3 - 1),
                        )
                    nc.vector.tensor_copy(
                        o_sb[:, ft * sw:(ft + 1) * sw], p3[:],
                    )
                    if ct == EPC - 1:
                        r0 = ct * OUT + ft * 128
                        nc.sync.dma_start(
                            out_d[r0:r0 + 128, 0:sw],
                            o_sb[:, ft * sw:(ft + 1) * sw],
                        )
                    elif ft % 2 == 1:
                        q = ft // 2
                        r0 = ct * OUT + q * 256
                        nc.sync.dma_start(
                            out_d[r0:r0 + 256, 0:sw].rearrange(
                                "(f p) c -> p f c", p=128),
                            o_sb[:, (ft - 1) * sw:(ft + 1) * sw].rearrange(
                                "p (f c) -> p f c", f=2),
                        )

    nc.compile()
    return nc


_NC_CACHE = {}


def _get_nc(slots):
    key = tuple(slots)
    if key not in _NC_CACHE:
        _NC_CACHE[key] = _build_bass(key)
    return _NC_CACHE[key]


def _plan(gid):
    """Balanced expert->(core,slot) assignment with shared slot widths."""
    counts = np.bincount(gid, minlength=G)
    rank = np.argsort(-counts, kind="stable")       # experts by count desc
    # expert rank 8j+i -> core i, slot j
    expert_of = rank.reshape(EPC, N_CORES)          # [slot, core]
    S = [int(-(-int(counts[expert_of[j]].max()) // 2) * 2) for j in range(EPC)]
    OFF = np.concatenate([[0], np.cumsum(S)]).astype(np.int64)
    cols = int(OFF[-1])
    order = np.full(N_CORES * cols, -1, dtype=np.int64)
    for j in range(EPC):
        for i in range(N_CORES):
            e = expert_of[j, i]
            idx = np.nonzero(gid == e)[0]
            base = i * cols + OFF[j]
            order[base: base + len(idx)] = idx
    return S, OFF, cols, expert_of, order


def _in_maps(classification_input, W1s, b1s, W1g, b1g, W2, b2, W3,
             order, cols, expert_of):
    x = np.asarray(classification_input, dtype=np.float32)
    valid = order >= 0
    x_perm = np.zeros((N_CORES * cols, IN_DIM), dtype=np.float32)
    x_perm[valid] = x[order[valid]]

    w1sT = np.ascontiguousarray(np.asarray(W1s, np.float32).T).astype(NP_BF16)
    w2T = np.ascontiguousarray(np.asarray(W2, np.float32).T).astype(NP_BF16)
    w3T = np.ascontiguousarray(np.asarray(W3, np.float32).T).astype(NP_BF16)
    b1s_t = np.asarray(b1s, np.float32).reshape(FT1, 128).T
    b2_t = np.asarray(b2, np.float32).reshape(FT2, 128).T
    # per-expert transposed: [G][IN_DIM][D]
    w1gT = np.ascontiguousarray(
        np.transpose(np.asarray(W1g, np.float32).reshape(G, D, IN_DIM), (0, 2, 1))
    )
    b1g_full = np.asarray(b1g, np.float32).reshape(G, FTG, 128)

    in_maps = []
    for c in range(N_CORES):
        experts_c = [int(expert_of[j, c]) for j in range(EPC)]
        xT_c = np.ascontiguousarray(
            x_perm[c * cols:(c + 1) * cols].T
        ).astype(NP_BF16)
        w1g_c = w1gT[experts_c].reshape(EPC * IN_DIM, D).astype(NP_BF16)
        b1g_c = np.transpose(
            b1g_full[experts_c], (2, 0, 1)).reshape(128, EPC * FTG)
        bias_c = np.ascontiguousarray(
            np.concatenate([b1s_t, b1g_c, b2_t], axis=1))
        in_maps.append({
            "x": xT_c,
            "w1s": w1sT,
            "w1g": w1g_c,
            "w2": w2T,
            "w3": w3T,
            "bias": bias_c,
        })
    return in_maps


def _assemble(results, order, S, OFF, cols):
    smax = S[0]
    out = np.empty((N, OUT), dtype=np.float32)
    for c, r in enumerate(results):
        a = r["out"].reshape(EPC, OUT, smax)        # [ct, feat, col]
        for j in range(EPC):
            blk = a[j, :, :S[j]].T                  # [S[j], OUT]
            o = order[c * cols + int(OFF[j]): c * cols + int(OFF[j]) + S[j]]
            v = o >= 0
            out[o[v]] = blk[v]
    return out


def run(trace=False, **inputs):
    gid = np.asarray(inputs["gate_ids"]).reshape(-1).astype(np.int64)
    S, OFF, cols, expert_of, order = _plan(gid)
    in_maps = _in_maps(
        inputs["classification_input"],
        inputs["W1s"], inputs["b1s"], inputs["W1g"], inputs["b1g"],
        inputs["W2"], inputs["b2"], inputs["W3"],
        order, cols, expert_of,
    )
    nc = _get_nc(S)
    res = run_bass_kernel_spmd(nc, in_maps, list(range(N_CORES)), trace=trace)
    out = _assemble(res.results, order, S, OFF, cols)
    return out, res


def kernel(**inputs):
    out, _ = run(trace=False, **inputs)
    return out


# revision 13
# speedup vs baseline: 1.0115x; 1.0115x over previous
"""Trainium2 Bass kernel for nn_GatedMLPConcat (MoE-routed gated MLP).

Math (reference):
  out_straight = relu(x @ W1s.T + b1s)                    # [N, 1024]
  out_gated    = relu(x @ W1g[gid].T + b1g[gid])          # [N, 512]  (only the selected expert matters)
  h  = relu(concat([out_straight, out_gated]) @ W2.T + b2)  # [N, 512]
  out = h @ W3.T                                           # [N, 1024]

Strategy:
  - Host-side MoE routing: group samples by gate_id into dense per-expert
    column blocks, avoiding the reference's 32x wasted expert FLOPs.
  - Load-balanced expert slots: experts sorted by sample count; rank 8j+i
    goes to core i, slot j, so all 8 cores share identical slot widths
    S[j] = max count in octile j (~[296,264,256,248] for seed 0, ~4% padding
    vs 16% for a uniform capacity). SPMD: one program, per-core weight data.
  - Everything on device is feature-major ([feature, sample] = matmul's
    [M partition, N free]); the host pre-transposes weights and activations.
  - bf16 matmul operands, f32 PSUM accumulation, f32 output.
  - DMAs in consumption order on the sync FIFO ring: x(kt0,slot0) first so
    the first matmul can issue ~1.5us after DMA start, then x/w1s per-kt
    interleaved, w1g per expert, w2, w3. Bias rides the scalar-engine ring
    (a tiny descriptor-bound transfer would stall the main ring).
  - L2/L3 interleaved per column slot; output DMA per quarter overlaps
    compute and keeps the tail short.
"""

import numpy as np

import concourse.bacc as bacc
import concourse.bass as bass
import concourse.mybir as mybir
from concourse import tile
from concourse.bass_utils import run_bass_kernel_spmd

# Problem constants (hardcoded per contract)
N = 8192
IN_DIM = 1024
D = 512
G = 32
OUT = 1024
SM, GM = 2, 1
SD = SM * D          # 1024 straight features
CD = (SM + GM) * D   # 1536 concat features

N_CORES = 8
EPC = G // N_CORES   # 4 expert slots per core

KT1 = IN_DIM // 128  # 8  k-tiles for layer 1
FT1 = SD // 128      # 8  straight feature tiles
FTG = D // 128       # 4  gated feature tiles (per expert)
KT2 = CD // 128      # 12 k-tiles for layer 2
FT2 = D // 128       # 4  layer-2 feature tiles
KT3 = D // 128       # 4  k-tiles for layer 3
FT3 = OUT // 128     # 8  layer-3 feature tiles

BF16 = mybir.dt.bfloat16
F32 = mybir.dt.float32
NP_BF16 = mybir.dt.np(BF16)


def _build_bass(slots):
    S = list(slots)            # per-slot column widths, identical on all cores
    OFF = [0]
    for s in S:
        OFF.append(OFF[-1] + s)
    cols = OFF[-1]
    smax = S[0]

    nc = bacc.Bacc()

    x_d = nc.dram_tensor("x", [IN_DIM, cols], BF16, kind="ExternalInput")
    w1s_d = nc.dram_tensor("w1s", [IN_DIM, SD], BF16, kind="ExternalInput")
    # expert-slot-major: [e][kt][128][D]
    w1g_d = nc.dram_tensor("w1g", [EPC * IN_DIM, D], BF16, kind="ExternalInput")
    w2_d = nc.dram_tensor("w2", [CD, D], BF16, kind="ExternalInput")
    w3_d = nc.dram_tensor("w3", [D, OUT], BF16, kind="ExternalInput")
    bias_d = nc.dram_tensor("bias", [128, FT1 + EPC * FTG + FT2], F32,
                            kind="ExternalInput")
    # [ct][ft][128][smax]; cols beyond S[ct] unwritten
    out_d = nc.dram_tensor("out", [EPC * OUT, smax], F32, kind="ExternalOutput")

    with tile.TileContext(nc) as tc:
        with (
            tc.tile_pool(name="acts", bufs=1) as acts,
            tc.tile_pool(name="weights", bufs=1) as weights,
            tc.tile_pool(name="outp", bufs=3) as outp,
            tc.tile_pool(name="psum", bufs=8, space="PSUM") as psum,
        ):
            x_sb = acts.tile([128, KT1 * cols], BF16)       # x[kt][:, col]
            h1_sb = acts.tile([128, KT2 * cols], BF16)      # concat acts
            h2_sb = acts.tile([128, KT3 * cols], BF16)
            w1s_sb = weights.tile([128, KT1 * SD], BF16)    # w1s[kt][:, feat]
            w1g_sb = weights.tile([128, EPC * KT1 * D], BF16)  # [e][kt][feat]
            w2_sb = weights.tile([128, KT2 * D], BF16)
            w3_sb = weights.tile([128, KT3 * OUT], BF16)
            bias_sb = weights.tile([128, FT1 + EPC * FTG + FT2], F32)

            # --- PE prewarm: junk matmuls with no DMA deps keep the PE HAM
            # activity window busy during the input lead-in, so real matmuls
            # start at 2.4GHz instead of the cold 1.2GHz gate
            warm_sb = weights.tile([128, 128], BF16)
            nc.vector.memset(warm_sb[:], 0.0)
            for w in range(24):
                wp = psum.tile([128, 128], F32, tag="ps", name=f"warm{w}")
                nc.tensor.matmul(wp[:], warm_sb[:], warm_sb[:],
                                 start=True, stop=True)

            # --- DMAs in consumption order (FIFO ring on sync) ---
            nc.scalar.dma_start(bias_sb[:], bias_d[:])
            # first matmul needs only x(kt0, slot0) + w1s(kt0, ft0)
            nc.sync.dma_start(x_sb[:, 0:S[0]], x_d[0:128, 0:S[0]])
            nc.sync.dma_start(w1s_sb[:, 0:128], w1s_d[0:128, 0:128])
            nc.sync.dma_start(w1s_sb[:, 128:SD], w1s_d[0:128, 128:])
            nc.sync.dma_start(x_sb[:, S[0]:cols], x_d[0:128, S[0]:])
            for kt in range(1, KT1):
                nc.sync.dma_start(
                    x_sb[:, kt * cols:(kt + 1) * cols],
                    x_d[kt * 128:(kt + 1) * 128, :],
                )
                nc.sync.dma_start(
                    w1s_sb[:, kt * SD:(kt + 1) * SD],
                    w1s_d[kt * 128:(kt + 1) * 128, :],
                )
            for e in range(EPC):
                nc.sync.dma_start(
                    w1g_sb[:, e * KT1 * D:(e + 1) * KT1 * D].rearrange(
                        "p (k c) -> p k c", k=KT1),
                    w1g_d[e * IN_DIM:(e + 1) * IN_DIM, :].rearrange(
                        "(k p) c -> p k c", p=128),
                )
            nc.sync.dma_start(
                w2_sb[:].rearrange("p (k c) -> p k c", k=KT2),
                w2_d[:].rearrange("(k p) c -> p k c", p=128))
            nc.sync.dma_start(
                w3_sb[:].rearrange("p (k c) -> p k c", k=KT3),
                w3_d[:].rearrange("(k p) c -> p k c", p=128))

            def xs(kt, ct):
                return x_sb[:, kt * cols + OFF[ct]: kt * cols + OFF[ct + 1]]

            def h1s(ft, ct):
                return h1_sb[:, ft * cols + OFF[ct]: ft * cols + OFF[ct + 1]]

            def h2s(ft, ct):
                return h2_sb[:, ft * cols + OFF[ct]: ft * cols + OFF[ct + 1]]

            # ---- Layer 1, straight path: h1[0:8] = relu(W1s @ x + b1s)
            for ft in range(FT1):
                ps = [psum.tile([128, S[i]], F32, tag="ps", name=f"ps{ft}_{i}")
                      for i in range(EPC)]
                for kt in range(KT1):
                    w_ap = w1s_sb[:, kt * SD + ft * 128: kt * SD + (ft + 1) * 128]
                    for ct in range(EPC):
                        nc.tensor.matmul(
                            ps[ct][:], w_ap, xs(kt, ct),
                            start=(kt == 0), stop=(kt == KT1 - 1),
                        )
                for ct in range(EPC):
                    nc.scalar.activation(
                        h1s(ft, ct), ps[ct][:],
                        mybir.ActivationFunctionType.Relu,
                        bias=bias_sb[:, ft:ft + 1],
                    )

            # ---- Layer 1, gated path: slot e columns get expert e's features
            for e in range(EPC):
                for ft in range(FTG):
                    p = psum.tile([128, S[e]], F32, tag="ps", name=f"psg{e}_{ft}")
                    for kt in range(KT1):
                        col0 = e * KT1 * D + kt * D + ft * 128
                        w_ap = w1g_sb[:, col0: col0 + 128]
                        nc.tensor.matmul(
                            p[:], w_ap, xs(kt, e),
                            start=(kt == 0), stop=(kt == KT1 - 1),
                        )
                    nc.scalar.activation(
                        h1s(FT1 + ft, e), p[:],
                        mybir.ActivationFunctionType.Relu,
                        bias=bias_sb[:, FT1 + e * FTG + ft:
                                     FT1 + e * FTG + ft + 1],
                    )

            # ---- Layers 2+3 interleaved per column slot; out DMA per quarter
            for ct in range(EPC):
                sw = S[ct]
                ps2 = [psum.tile([128, sw], F32, tag="ps", name=f"ps2_{ct}_{i}")
                       for i in range(FT2)]
                for kt in range(KT2):
                    for ft in range(FT2):
                        w_ap = w2_sb[:, kt * D + ft * 128: kt * D + (ft + 1) * 128]
                        nc.tensor.matmul(
                            ps2[ft][:], w_ap, h1s(kt, ct),
                            start=(kt == 0), stop=(kt == KT2 - 1),
                        )
                for ft in range(FT2):
                    nc.scalar.activation(
                        h2s(ft, ct), ps2[ft][:],
                        mybir.ActivationFunctionType.Relu,
                        bias=bias_sb[:, FT1 + EPC * FTG + ft:
                                     FT1 + EPC * FTG + ft + 1],
                    )

                o_sb = outp.tile([128, FT3 * sw], F32, tag="o", name=f"o{ct}")
                for ft in range(FT3):
                    p3 = psum.tile([128, sw], F32, tag="ps", name=f"ps3_{ct}_{ft}")
                    for kt in range(KT3):
                        w_ap = w3_sb[:, kt * OUT + ft * 128: kt * OUT + (ft + 1) * 128]
                        nc.tensor.matmul(
                            p3[:], w_ap, h2s(kt, ct),
                            start=(kt == 0), stop=(kt == KT3 - 1),
                        )
                    nc.vector.tensor_copy(
                        o_sb[:, ft * sw:(ft + 1) * sw], p3[:],
                    )
                    if ft % 2 == 1:
                        q = ft // 2
                        r0 = ct * OUT + q * 256
                        nc.sync.dma_start(
                            out_d[r0:r0 + 256, 0:sw].rearrange(
                                "(f p) c -> p f c", p=128),
                            o_sb[:, (ft - 1) * sw:(ft + 1) * sw].rearrange(
                                "p (f c) -> p f c", f=2),
                        )

    nc.compile()
    return nc


_NC_CACHE = {}


def _get_nc(slots):
    key = tuple(slots)
    if key not in _NC_CACHE:
        _NC_CACHE[key] = _build_bass(key)
    return _NC_CACHE[key]


def _plan(gid):
    """Balanced expert->(core,slot) assignment with shared slot widths."""
    counts = np.bincount(gid, minlength=G)
    rank = np.argsort(-counts, kind="stable")       # experts by count desc
    # expert rank 8j+i -> core i, slot j
    expert_of = rank.reshape(EPC, N_CORES)          # [slot, core]
    S = [int(-(-int(counts[expert_of[j]].max()) // 2) * 2) for j in range(EPC)]
    OFF = np.concatenate([[0], np.cumsum(S)]).astype(np.int64)
    cols = int(OFF[-1])
    order = np.full(N_CORES * cols, -1, dtype=np.int64)
    for j in range(EPC):
        for i in range(N_CORES):
            e = expert_of[j, i]
            idx = np.nonzero(gid == e)[0]
            base = i * cols + OFF[j]
            order[base: base + len(idx)] = idx
    return S, OFF, cols, expert_of, order


def _in_maps(classification_input, W1s, b1s, W1g, b1g, W2, b2, W3,
             order, cols, expert_of):
    x = np.asarray(classification_input, dtype=np.float32)
    valid = order >= 0
    x_perm = np.zeros((N_CORES * cols, IN_DIM), dtype=np.float32)
    x_perm[valid] = x[order[valid]]

    w1sT = np.ascontiguousarray(np.asarray(W1s, np.float32).T).astype(NP_BF16)
    w2T = np.ascontiguousarray(np.asarray(W2, np.float32).T).astype(NP_BF16)
    w3T = np.ascontiguousarray(np.asarray(W3, np.float32).T).astype(NP_BF16)
    b1s_t = np.asarray(b1s, np.float32).reshape(FT1, 128).T
    b2_t = np.asarray(b2, np.float32).reshape(FT2, 128).T
    # per-expert transposed: [G][IN_DIM][D]
    w1gT = np.ascontiguousarray(
        np.transpose(np.asarray(W1g, np.float32).reshape(G, D, IN_DIM), (0, 2, 1))
    )
    b1g_full = np.asarray(b1g, np.float32).reshape(G, FTG, 128)

    in_maps = []
    for c in range(N_CORES):
        experts_c = [int(expert_of[j, c]) for j in range(EPC)]
        xT_c = np.ascontiguousarray(
            x_perm[c * cols:(c + 1) * cols].T
        ).astype(NP_BF16)
        w1g_c = w1gT[experts_c].reshape(EPC * IN_DIM, D).astype(NP_BF16)
        b1g_c = np.transpose(
            b1g_full[experts_c], (2, 0, 1)).reshape(128, EPC * FTG)
        bias_c = np.ascontiguousarray(
            np.concatenate([b1s_t, b1g_c, b2_t], axis=1))
        in_maps.append({
            "x": xT_c,
            "w1s": w1sT,
            "w1g": w1g_c,
            "w2": w2T,
            "w3": w3T,
            "bias": bias_c,
        })
    return in_maps


def _assemble(results, order, S, OFF, cols):
    smax = S[0]
    out = np.empty((N, OUT), dtype=np.float32)
    for c, r in enumerate(results):
        a = r["out"].reshape(EPC, OUT, smax)        # [ct, feat, col]
        for j in range(EPC):
            blk = a[j, :, :S[j]].T                  # [S[j], OUT]
            o = order[c * cols + int(OFF[j]): c * cols + int(OFF[j]) + S[j]]
            v = o >= 0
            out[o[v]] = blk[v]
    return out


def run(trace=False, **inputs):
    gid = np.asarray(inputs["gate_ids"]).reshape(-1).astype(np.int64)
    S, OFF, cols, expert_of, order = _plan(gid)
    in_maps = _in_maps(
        inputs["classification_input"],
        inputs["W1s"], inputs["b1s"], inputs["W1g"], inputs["b1g"],
        inputs["W2"], inputs["b2"], inputs["W3"],
        order, cols, expert_of,
    )
    nc = _get_nc(S)
    res = run_bass_kernel_spmd(nc, in_maps, list(range(N_CORES)), trace=trace)
    out = _assemble(res.results, order, S, OFF, cols)
    return out, res


def kernel(**inputs):
    out, _ = run(trace=False, **inputs)
    return out
